# revision 2
# baseline (speedup 1.0000x reference)
"""Trainium2 Bass kernel for nn_Bessel: out = i0e(z) * exp(z - 2a), z = 2a*sqrt((1+x@yT)/2), a=10.

Mode "delta" (current): single ACT pass + DVE bitcast-exp.

Math: ln out ~= p + r*sqrt(SQ*c + SB)  (4-param minimax fit, c = x@yT).
Fold r into the sqrt:  zw = sqrt(r^2*SQ*c + r^2*SB),  out = exp(zw + p).

Per core (row-shard of x, y replicated; out tile [1024, 8192]):

  PE:  c into PSUM as [xh;xl] @ [yh;yh]  (bf16 split of x only; K=128,
       4 matmuls of 512 cols per 2048-col PSUM tile)
  ACT: zw = Sqrt(scale*c + bias) evacuating PSUM -> fp16 zw   [the ONLY ACT
       pass; sqrt table stays loaded -- no table switches at all]
  DVE: exp via fp16-Schraudolph bitcast:  i16 = cvt(K*zw + B) is the fp16
       bit pattern of 2^v*(1+eps(frac)), v = log2e*(zw+p) + S  (S=16 output
       prescale keeps everything fp16-normal; host multiplies by 2^-16).
       tensor_scalar f16->i16 runs in 4x DVE mode (~2.2us per m-tile).
       To kill the +-3% PL mantissa error, "avg" m-tiles compute two
       half-octave-shifted variants and blend:
           out = f16bits(i(v-1)) + 0.70710678 * f16bits(i(v-0.5))
       (the -1024/-512 bias shifts are exact exponent shifts; the blend is
       one scalar_tensor_tensor, 2x mode) -> ~4x lower error. 2 of 8
       m-tiles stay "plain" (single i16 pass) to keep DVE (<57us) under
       ACT (~59us).
  DMA: fp16 out -> HBM (16MB/core; host upcasts and scales).

Engine budget per core/iter: ACT ~59us (bottleneck), DVE ~57us, PE 28-55us,
DMA out ~47us. Predicted L2 rel err ~1.0e-2 (gate 2e-2): fit 2.3e-3 +
fp16 zw 3.3e-3 + avg-Schraudolph 4.1e-3 on 6/8 rows + plain 1.8e-2 on 2/8.

Mode "gamma" kept for A/B: 2 ACT passes (Sqrt + Exp), bf16 out, 132us.
"""

import contextlib

import numpy as np

import concourse.bacc as bacc
import concourse.mybir as mybir
from concourse.tile import TileContext
from concourse.tile_autobufs import add_dep_helper
from concourse.bass_utils import run_bass_kernel_spmd

AF = mybir.ActivationFunctionType
OP = mybir.AluOpType
F32 = mybir.dt.float32
F16 = mybir.dt.float16
I16 = mybir.dt.int16
BF16 = mybir.dt.bfloat16
BFNP = mybir.dt.np(BF16)

N_CORES = 8
N_ROWS, M_COLS, DIM = 8192, 8192, 64
ROWS = N_ROWS // N_CORES          # 1024 rows of x per core
MTILES = ROWS // 128              # 8 partition tiles per core

# minimax fit of the whole exponent: ln out = GAM_P + GAM_R*sqrt(a*u + b),
# u = 200c+200 in [50, 368]; max abs err on the exponent 2.12e-3.
GAM_P = -22.179313758272478
GAM_R = 0.7814668006400919
GAM_SQ_SCALE = 314.6476142409728          # 200*a
GAM_SQ_BIAS = 325.04327804569425          # 200*a + b

LOG2E = 1.4426950408889634
OUT_EXP = 16                              # device out = true out * 2^OUT_EXP
SIG_AVG = 61.25                           # Schraudolph constants (calibrated
SIG_PLAIN = 62.25                         # numerically on the real data)

# delta: fold GAM_R into the sqrt
D_SQ_SCALE = GAM_R * GAM_R * GAM_SQ_SCALE
D_SQ_BIAS = GAM_R * GAM_R * GAM_SQ_BIAS
D_K = 1024.0 * LOG2E
# i(w) = cvt(1024*w + 1024*15 - sigma) = fp16 bits of ~2^w;  v = LOG2E*zw + PB
D_PB = GAM_P * LOG2E + OUT_EXP

MODE = "delta"
PLAIN_TILES = (3, 7)                      # m-tiles using single-pass exp

_cache = {}


def _build_delta(psum_fd=2048, mm_fd=512, zw_bufs=3, i_bufs=4, o_bufs=2,
                 plain=PLAIN_TILES, iters=1):
    nc = bacc.Bacc(None, target_bir_lowering=False)
    xs_d = nc.dram_tensor("xs", [2 * DIM, ROWS], BF16, kind="ExternalInput")
    ys_d = nc.dram_tensor("ys", [2 * DIM, M_COLS], BF16, kind="ExternalInput")
    out_d = nc.dram_tensor("out", [ROWS, M_COLS], F16, kind="ExternalOutput")

    b_base = (D_PB + 15.0) * 1024.0
    b_avg1 = b_base - SIG_AVG - 1024.0    # i(v-1)
    b_avg2 = b_base - SIG_AVG - 512.0     # i(v-0.5)
    b_plain = b_base - SIG_PLAIN          # i(v)

    with TileContext(nc) as tc:
        with (
            tc.tile_pool(name="inp", bufs=1) as inp,
            tc.tile_pool(name="consts", bufs=1) as consts,
            tc.tile_pool(name="zw", bufs=zw_bufs) as zwpool,
            tc.tile_pool(name="i16", bufs=i_bufs) as ipool,
            tc.tile_pool(name="obf", bufs=o_bufs) as obfpool,
            tc.tile_pool(name="psum", bufs=4096 // psum_fd, space="PSUM") as psum,
        ):
            xs = inp.tile([2 * DIM, ROWS], BF16)
            ys = inp.tile([2 * DIM, M_COLS], BF16)
            nc.sync.dma_start(out=xs[:], in_=xs_d[:])
            for q in range(0, M_COLS, 2048):
                nc.sync.dma_start(out=ys[:, q:q + 2048], in_=ys_d[:, q:q + 2048])

            bsq = consts.tile([128, 1], F32)
            nc.gpsimd.memset(bsq[:], float(D_SQ_BIAS))

            nchunk = M_COLS // psum_fd
            loop_cm = tc.For_i(0, iters) if iters > 1 else contextlib.nullcontext(0)
            with loop_cm as _i:
                for m in range(MTILES):
                    msl = slice(m * 128, (m + 1) * 128)
                    zw = zwpool.tile([128, M_COLS], F16, tag="zw")
                    for nb in range(nchunk):
                        pt = psum.tile([128, psum_fd], F32, tag="ps")
                        for j in range(psum_fd // mm_fd):
                            col = nb * psum_fd + j * mm_fd
                            nc.tensor.matmul(
                                pt[:, j * mm_fd:(j + 1) * mm_fd],
                                xs[:, msl], ys[:, col:col + mm_fd],
                                start=True, stop=True,
                            )
                        sl = slice(nb * psum_fd, (nb + 1) * psum_fd)
                        nc.scalar.activation(
                            zw[:, sl], pt[:], AF.Sqrt,
                            bias=bsq[:], scale=float(D_SQ_SCALE),
                        )
                    osl = slice(m * 128, (m + 1) * 128)
                    if m in plain:
                        i1 = ipool.tile([128, M_COLS], I16, tag="i16")
                        nc.vector.tensor_scalar(
                            out=i1[:], in0=zw[:],
                            scalar1=float(D_K), scalar2=float(b_plain),
                            op0=OP.mult, op1=OP.add,
                        )
                        nc.sync.dma_start(out=out_d[osl, :], in_=i1[:].bitcast(F16))
                    else:
                        i1 = ipool.tile([128, M_COLS], I16, tag="i16")
                        i2 = ipool.tile([128, M_COLS], I16, tag="i16")
                        nc.vector.tensor_scalar(
                            out=i1[:], in0=zw[:],
                            scalar1=float(D_K), scalar2=float(b_avg1),
                            op0=OP.mult, op1=OP.add,
                        )
                        nc.vector.tensor_scalar(
                            out=i2[:], in0=zw[:],
                            scalar1=float(D_K), scalar2=float(b_avg2),
                            op0=OP.mult, op1=OP.add,
                        )
                        obf = obfpool.tile([128, M_COLS], F16, tag="obf")
                        nc.vector.scalar_tensor_tensor(
                            obf[:], i2[:].bitcast(F16), 0.70710678,
                            i1[:].bitcast(F16), OP.mult, OP.add,
                        )
                        nc.sync.dma_start(out=out_d[osl, :], in_=obf[:])

    nc.finalize()
    return nc


def _build_gamma(group=8, psum_fd=1024, obf_bufs=3, zw_extra=0, exp_split=1,
                 iters=1, zw_fp16=True, no_yl=True, mm_fd=512):
    """mode gamma (baseline): 2 ACT passes, table-set batched."""
    nc = bacc.Bacc(None, target_bir_lowering=False)
    xs_d = nc.dram_tensor("xs", [2 * DIM, ROWS], BF16, kind="ExternalInput")
    ys_d = nc.dram_tensor("ys", [2 * DIM, M_COLS], BF16, kind="ExternalInput")
    out_d = nc.dram_tensor("out", [ROWS, M_COLS], BF16, kind="ExternalOutput")

    zw_dt = mybir.dt.float16 if zw_fp16 else F32

    with TileContext(nc) as tc:
        with (
            tc.tile_pool(name="inp", bufs=1) as inp,
            tc.tile_pool(name="consts", bufs=1) as consts,
            tc.tile_pool(name="zw", bufs=group + zw_extra) as zwpool,
            tc.tile_pool(name="obf", bufs=obf_bufs) as obfpool,
            tc.tile_pool(name="psum", bufs=4096 // psum_fd, space="PSUM") as psum,
        ):
            xs = inp.tile([2 * DIM, ROWS], BF16)
            ys = inp.tile([2 * DIM, M_COLS], BF16)
            nc.sync.dma_start(out=xs[:], in_=xs_d[:])
            for q in range(0, M_COLS, 2048):
                nc.sync.dma_start(out=ys[:, q:q + 2048], in_=ys_d[:, q:q + 2048])

            bsq = consts.tile([128, 1], F32)
            nc.gpsimd.memset(bsq[:], float(GAM_SQ_BIAS))
            bexp = consts.tile([128, 1], F32)
            nc.gpsimd.memset(bexp[:], float(GAM_P))

            nchunk = M_COLS // psum_fd
            mtile_groups = [
                list(range(g, min(g + group, MTILES)))
                for g in range(0, MTILES, group)
            ]
            loop_cm = tc.For_i(0, iters) if iters > 1 else contextlib.nullcontext(0)
            with loop_cm as _i:
                last_exp = None
                for grp in mtile_groups:
                    zw_tiles = {}
                    last_evac = None
                    for m in grp:
                        zw = zwpool.tile([128, M_COLS], zw_dt, tag="zw")
                        zw_tiles[m] = zw
                        msl = slice(m * 128, (m + 1) * 128)
                        for nb in range(nchunk):
                            pt = psum.tile([128, psum_fd], F32, tag="ps")
                            for j in range(psum_fd // mm_fd):
                                col = nb * psum_fd + j * mm_fd
                                jsl = slice(j * mm_fd, (j + 1) * mm_fd)
                                nc.tensor.matmul(
                                    pt[:, jsl],
                                    xs[:, msl], ys[:, col:col + mm_fd],
                                    start=True, stop=True,
                                )
                            sl = slice(nb * psum_fd, (nb + 1) * psum_fd)
                            ev = nc.scalar.activation(
                                zw[:, sl], pt[:], AF.Sqrt,
                                bias=bsq[:], scale=float(GAM_SQ_SCALE)
                            )
                            if last_exp is not None:
                                add_dep_helper(
                                    ev.ins, last_exp.ins, sync=False,
                                    reason="batch sqrt after prev group exp",
                                )
                            last_evac = ev
                    for m in grp:
                        zw = zw_tiles[m]
                        efd = M_COLS // exp_split
                        obf = obfpool.tile([128, M_COLS], BF16, tag="obf")
                        for e in range(exp_split):
                            esl = slice(e * efd, (e + 1) * efd)
                            exp_inst = nc.scalar.activation(
                                obf[:, esl], zw[:, esl], AF.Exp,
                                bias=bexp[:], scale=float(GAM_R)
                            )
                            add_dep_helper(
                                exp_inst.ins, last_evac.ins, sync=False,
                                reason="batch exp after group sqrt (table switch)",
                            )
                            last_exp = exp_inst
                            nc.sync.dma_start(
                                out=out_d[m * 128:(m + 1) * 128, esl],
                                in_=obf[:, esl],
                            )

    nc.finalize()
    return nc


def _build(mode, iters=1, **kw):
    if mode == "delta":
        return _build_delta(iters=iters, **kw)
    return _build_gamma(iters=iters, **kw)


LAST_RESULTS = None


def _split_bf16(a):
    hi = a.astype(BFNP)
    lo = (a - hi.astype(np.float32)).astype(BFNP)
    return hi, lo


def make_in_maps(x, y):
    yT = y.T
    yh, _yl = _split_bf16(yT)
    ys = np.ascontiguousarray(np.concatenate([yh, yh], axis=0))
    in_maps = []
    for i in range(N_CORES):
        xT = x[i * ROWS:(i + 1) * ROWS].T
        xh, xl = _split_bf16(xT)
        xstack = np.ascontiguousarray(np.concatenate([xh, xl], axis=0))
        in_maps.append({"xs": xstack, "ys": ys})
    return in_maps


def kernel(x: np.ndarray, y: np.ndarray) -> np.ndarray:
    global LAST_RESULTS
    x = np.ascontiguousarray(x, dtype=np.float32)
    y = np.ascontiguousarray(y, dtype=np.float32)
    assert x.shape == (N_ROWS, DIM) and y.shape == (M_COLS, DIM)

    if MODE not in _cache:
        _cache[MODE] = _build(MODE)
    nc = _cache[MODE]

    in_maps = make_in_maps(x, y)

    LAST_RESULTS = run_bass_kernel_spmd(nc, in_maps, list(range(N_CORES)))
    out = np.concatenate([r["out"] for r in LAST_RESULTS.results], axis=0)
    if out.dtype == np.float16:
        out = out.astype(np.float32) * np.float32(2.0 ** -OUT_EXP)
    elif out.dtype != np.float32:
        out = out.astype(np.float32)
    return out


# revision 7
# speedup vs baseline: 1.1563x; 1.1563x over previous
"""Trainium2 Bass kernel for nn_Bessel: out = i0e(z) * exp(z - 2a), z = 2a*sqrt((1+x@yT)/2), a=10.

Mode "delta" (current): single ACT pass + DVE bitcast-exp.

Math: ln out ~= p + r*sqrt(SQ*c + SB)  (4-param minimax fit, c = x@yT).
Fold r into the sqrt:  zw = sqrt(r^2*SQ*c + r^2*SB),  out = exp(zw + p).

Per core (row-shard of x, y replicated; out tile [1024, 8192]):

  PE:  c into PSUM as [xh;xl] @ [yh;yh]  (bf16 split of x only; K=128,
       4 matmuls of 512 cols per 2048-col PSUM tile)
  ACT: zw = Sqrt(scale*c + bias) evacuating PSUM -> fp16 zw   [the ONLY ACT
       pass; sqrt table stays loaded -- no table switches at all]
  DVE: exp via fp16-Schraudolph bitcast:  i16 = cvt(K*zw + B) is the fp16
       bit pattern of 2^v*(1+eps(frac)), v = log2e*(zw+p) + S  (S=16 output
       prescale keeps everything fp16-normal; host multiplies by 2^-16).
       tensor_scalar f16->i16 runs in 4x DVE mode (~2.2us per m-tile).
       To kill the +-3% PL mantissa error, "avg" m-tiles compute two
       half-octave-shifted variants and blend:
           out = f16bits(i(v-1)) + 0.70710678 * f16bits(i(v-0.5))
       (the -1024/-512 bias shifts are exact exponent shifts; the blend is
       one scalar_tensor_tensor, 2x mode) -> ~4x lower error. 2 of 8
       m-tiles stay "plain" (single i16 pass) to keep DVE (<57us) under
       ACT (~59us).
  DMA: fp16 out -> HBM (16MB/core; host upcasts and scales).

Engine budget per core/iter: ACT ~59us (bottleneck), DVE ~57us, PE 28-55us,
DMA out ~47us. Predicted L2 rel err ~1.0e-2 (gate 2e-2): fit 2.3e-3 +
fp16 zw 3.3e-3 + avg-Schraudolph 4.1e-3 on 6/8 rows + plain 1.8e-2 on 2/8.

Mode "gamma" kept for A/B: 2 ACT passes (Sqrt + Exp), bf16 out, 132us.
"""

import contextlib

import numpy as np

import concourse.bacc as bacc
import concourse.mybir as mybir
from concourse.tile import TileContext
from concourse.tile_autobufs import add_dep_helper
from concourse.bass_utils import run_bass_kernel_spmd

AF = mybir.ActivationFunctionType
OP = mybir.AluOpType
F32 = mybir.dt.float32
F16 = mybir.dt.float16
I16 = mybir.dt.int16
BF16 = mybir.dt.bfloat16
BFNP = mybir.dt.np(BF16)

N_CORES = 8
N_ROWS, M_COLS, DIM = 8192, 8192, 64
ROWS = N_ROWS // N_CORES          # 1024 rows of x per core
MTILES = ROWS // 128              # 8 partition tiles per core

# minimax fit of the whole exponent: ln out = GAM_P + GAM_R*sqrt(a*u + b),
# u = 200c+200 in [50, 368]; max abs err on the exponent 2.12e-3.
GAM_P = -22.179313758272478
GAM_R = 0.7814668006400919
GAM_SQ_SCALE = 314.6476142409728          # 200*a
GAM_SQ_BIAS = 325.04327804569425          # 200*a + b

LOG2E = 1.4426950408889634
OUT_EXP = 16                              # device out = true out * 2^OUT_EXP
SIG_AVG = 61.75                           # Schraudolph constants (calibrated
SIG_PLAIN = 62.25                         # numerically on the real data)
# unequal-weight blend: out = f16bits(i(v-1+DLT)) + f16bits(i(v-1.5+DLT));
# DLT makes the nominal weights sum to 1, so the blend is a plain
# tensor_tensor ADD (2x DVE mode; scalar_tensor_tensor has NO fast uop).
BLEND_DLT = 0.22844669683638807           # -log2(2^-1 + 2^-1.5)

# delta: fold GAM_R into the sqrt
D_SQ_SCALE = GAM_R * GAM_R * GAM_SQ_SCALE
D_SQ_BIAS = GAM_R * GAM_R * GAM_SQ_BIAS
D_K = 1024.0 * LOG2E
# i(w) = cvt(1024*w + 1024*15 - sigma) = fp16 bits of ~2^w;  v = LOG2E*zw + PB
D_PB = GAM_P * LOG2E + OUT_EXP

MODE = "delta"
PLAIN_TILES = (3, 7)                      # m-tiles using single-pass exp

_cache = {}


def _build_delta(psum_fd=2048, mm_fd=512, zw_bufs=3, i_bufs=4, o_bufs=2,
                 plain=PLAIN_TILES, iters=1, unroll=1):
    nc = bacc.Bacc(None, target_bir_lowering=False)
    xs_d = nc.dram_tensor("xs", [2 * DIM, ROWS], BF16, kind="ExternalInput")
    ys_d = nc.dram_tensor("ys", [2 * DIM, M_COLS], BF16, kind="ExternalInput")
    out_d = nc.dram_tensor("out", [ROWS, M_COLS], F16, kind="ExternalOutput")

    b_base = (D_PB + 15.0) * 1024.0
    b_avg1 = b_base - SIG_AVG + 1024.0 * (BLEND_DLT - 1.0)   # i(v-1+DLT)
    b_avg2 = b_base - SIG_AVG + 1024.0 * (BLEND_DLT - 1.5)   # i(v-1.5+DLT)
    b_plain = b_base - SIG_PLAIN                             # i(v)

    with TileContext(nc) as tc:
        with (
            tc.tile_pool(name="inp", bufs=1) as inp,
            tc.tile_pool(name="consts", bufs=1) as consts,
            tc.tile_pool(name="zw", bufs=zw_bufs) as zwpool,
            tc.tile_pool(name="i16", bufs=i_bufs) as ipool,
            tc.tile_pool(name="obf", bufs=o_bufs) as obfpool,
            tc.tile_pool(name="psum", bufs=4096 // psum_fd, space="PSUM") as psum,
        ):
            xs = inp.tile([2 * DIM, ROWS], BF16)
            ys = inp.tile([2 * DIM, M_COLS], BF16)
            nc.sync.dma_start(out=xs[:], in_=xs_d[:])
            for q in range(0, M_COLS, 2048):
                nc.sync.dma_start(out=ys[:, q:q + 2048], in_=ys_d[:, q:q + 2048])

            bsq = consts.tile([128, 1], F32)
            nc.gpsimd.memset(bsq[:], float(D_SQ_BIAS))

            nchunk = M_COLS // psum_fd
            loop_cm = tc.For_i(0, iters) if iters > 1 else contextlib.nullcontext(0)
            with loop_cm as _i:
              for _u in range(unroll):
                for m in range(MTILES):
                    msl = slice(m * 128, (m + 1) * 128)
                    zw = zwpool.tile([128, M_COLS], F16, tag="zw")
                    for nb in range(nchunk):
                        pt = psum.tile([128, psum_fd], F32, tag="ps")
                        for j in range(psum_fd // mm_fd):
                            col = nb * psum_fd + j * mm_fd
                            nc.tensor.matmul(
                                pt[:, j * mm_fd:(j + 1) * mm_fd],
                                xs[:, msl], ys[:, col:col + mm_fd],
                                start=True, stop=True,
                            )
                        sl = slice(nb * psum_fd, (nb + 1) * psum_fd)
                        nc.scalar.activation(
                            zw[:, sl], pt[:], AF.Sqrt,
                            bias=bsq[:], scale=float(D_SQ_SCALE),
                        )
                    osl = slice(m * 128, (m + 1) * 128)
                    if m in plain:
                        i1 = ipool.tile([128, M_COLS], I16, tag="i16")
                        nc.vector.tensor_scalar(
                            out=i1[:], in0=zw[:],
                            scalar1=float(D_K), scalar2=float(b_plain),
                            op0=OP.mult, op1=OP.add,
                        )
                        nc.sync.dma_start(out=out_d[osl, :], in_=i1[:].bitcast(F16))
                    else:
                        i1 = ipool.tile([128, M_COLS], I16, tag="i16")
                        i2 = ipool.tile([128, M_COLS], I16, tag="i16")
                        nc.vector.tensor_scalar(
                            out=i1[:], in0=zw[:],
                            scalar1=float(D_K), scalar2=float(b_avg1),
                            op0=OP.mult, op1=OP.add,
                        )
                        nc.vector.tensor_scalar(
                            out=i2[:], in0=zw[:],
                            scalar1=float(D_K), scalar2=float(b_avg2),
                            op0=OP.mult, op1=OP.add,
                        )
                        obf = obfpool.tile([128, M_COLS], F16, tag="obf")
                        nc.vector.tensor_tensor(
                            obf[:], i1[:].bitcast(F16), i2[:].bitcast(F16),
                            OP.add,
                        )
                        nc.sync.dma_start(out=out_d[osl, :], in_=obf[:])

    nc.finalize()
    return nc


def _build_gamma(group=8, psum_fd=1024, obf_bufs=3, zw_extra=0, exp_split=1,
                 iters=1, zw_fp16=True, no_yl=True, mm_fd=512):
    """mode gamma (baseline): 2 ACT passes, table-set batched."""
    nc = bacc.Bacc(None, target_bir_lowering=False)
    xs_d = nc.dram_tensor("xs", [2 * DIM, ROWS], BF16, kind="ExternalInput")
    ys_d = nc.dram_tensor("ys", [2 * DIM, M_COLS], BF16, kind="ExternalInput")
    out_d = nc.dram_tensor("out", [ROWS, M_COLS], BF16, kind="ExternalOutput")

    zw_dt = mybir.dt.float16 if zw_fp16 else F32

    with TileContext(nc) as tc:
        with (
            tc.tile_pool(name="inp", bufs=1) as inp,
            tc.tile_pool(name="consts", bufs=1) as consts,
            tc.tile_pool(name="zw", bufs=group + zw_extra) as zwpool,
            tc.tile_pool(name="obf", bufs=obf_bufs) as obfpool,
            tc.tile_pool(name="psum", bufs=4096 // psum_fd, space="PSUM") as psum,
        ):
            xs = inp.tile([2 * DIM, ROWS], BF16)
            ys = inp.tile([2 * DIM, M_COLS], BF16)
            nc.sync.dma_start(out=xs[:], in_=xs_d[:])
            for q in range(0, M_COLS, 2048):
                nc.sync.dma_start(out=ys[:, q:q + 2048], in_=ys_d[:, q:q + 2048])

            bsq = consts.tile([128, 1], F32)
            nc.gpsimd.memset(bsq[:], float(GAM_SQ_BIAS))
            bexp = consts.tile([128, 1], F32)
            nc.gpsimd.memset(bexp[:], float(GAM_P))

            nchunk = M_COLS // psum_fd
            mtile_groups = [
                list(range(g, min(g + group, MTILES)))
                for g in range(0, MTILES, group)
            ]
            loop_cm = tc.For_i(0, iters) if iters > 1 else contextlib.nullcontext(0)
            with loop_cm as _i:
                last_exp = None
                for grp in mtile_groups:
                    zw_tiles = {}
                    last_evac = None
                    for m in grp:
                        zw = zwpool.tile([128, M_COLS], zw_dt, tag="zw")
                        zw_tiles[m] = zw
                        msl = slice(m * 128, (m + 1) * 128)
                        for nb in range(nchunk):
                            pt = psum.tile([128, psum_fd], F32, tag="ps")
                            for j in range(psum_fd // mm_fd):
                                col = nb * psum_fd + j * mm_fd
                                jsl = slice(j * mm_fd, (j + 1) * mm_fd)
                                nc.tensor.matmul(
                                    pt[:, jsl],
                                    xs[:, msl], ys[:, col:col + mm_fd],
                                    start=True, stop=True,
                                )
                            sl = slice(nb * psum_fd, (nb + 1) * psum_fd)
                            ev = nc.scalar.activation(
                                zw[:, sl], pt[:], AF.Sqrt,
                                bias=bsq[:], scale=float(GAM_SQ_SCALE)
                            )
                            if last_exp is not None:
                                add_dep_helper(
                                    ev.ins, last_exp.ins, sync=False,
                                    reason="batch sqrt after prev group exp",
                                )
                            last_evac = ev
                    for m in grp:
                        zw = zw_tiles[m]
                        efd = M_COLS // exp_split
                        obf = obfpool.tile([128, M_COLS], BF16, tag="obf")
                        for e in range(exp_split):
                            esl = slice(e * efd, (e + 1) * efd)
                            exp_inst = nc.scalar.activation(
                                obf[:, esl], zw[:, esl], AF.Exp,
                                bias=bexp[:], scale=float(GAM_R)
                            )
                            add_dep_helper(
                                exp_inst.ins, last_evac.ins, sync=False,
                                reason="batch exp after group sqrt (table switch)",
                            )
                            last_exp = exp_inst
                            nc.sync.dma_start(
                                out=out_d[m * 128:(m + 1) * 128, esl],
                                in_=obf[:, esl],
                            )

    nc.finalize()
    return nc


def _build(mode, iters=1, **kw):
    if mode == "delta":
        return _build_delta(iters=iters, **kw)
    return _build_gamma(iters=iters, **kw)


LAST_RESULTS = None


def _split_bf16(a):
    hi = a.astype(BFNP)
    lo = (a - hi.astype(np.float32)).astype(BFNP)
    return hi, lo


def make_in_maps(x, y):
    yT = y.T
    yh, _yl = _split_bf16(yT)
    ys = np.ascontiguousarray(np.concatenate([yh, yh], axis=0))
    in_maps = []
    for i in range(N_CORES):
        xT = x[i * ROWS:(i + 1) * ROWS].T
        xh, xl = _split_bf16(xT)
        xstack = np.ascontiguousarray(np.concatenate([xh, xl], axis=0))
        in_maps.append({"xs": xstack, "ys": ys})
    return in_maps


def kernel(x: np.ndarray, y: np.ndarray) -> np.ndarray:
    global LAST_RESULTS
    x = np.ascontiguousarray(x, dtype=np.float32)
    y = np.ascontiguousarray(y, dtype=np.float32)
    assert x.shape == (N_ROWS, DIM) and y.shape == (M_COLS, DIM)

    if MODE not in _cache:
        _cache[MODE] = _build(MODE)
    nc = _cache[MODE]

    in_maps = make_in_maps(x, y)

    LAST_RESULTS = run_bass_kernel_spmd(nc, in_maps, list(range(N_CORES)))
    out = np.concatenate([r["out"] for r in LAST_RESULTS.results], axis=0)
    if out.dtype == np.float16:
        out = out.astype(np.float32) * np.float32(2.0 ** -OUT_EXP)
    elif out.dtype != np.float32:
        out = out.astype(np.float32)
    return out


# revision 14
# speedup vs baseline: 1.4497x; 1.2537x over previous
"""Trainium2 Bass kernel for nn_Bessel: out = i0e(z) * exp(z - 2a), z = 2a*sqrt((1+x@yT)/2), a=10.

Mode "delta" (current): single ACT pass + DVE bitcast-exp.

Math: ln out ~= p + r*sqrt(SQ*c + SB)  (4-param minimax fit, c = x@yT).
Fold r into the sqrt:  zw = sqrt(r^2*SQ*c + r^2*SB),  out = exp(zw + p).

Per core (row-shard of x, y replicated; out tile [1024, 8192]):

  PE:  c into PSUM as [xh;xl] @ [yh;yh]  (bf16 split of x only; K=128,
       4 matmuls of 512 cols per 2048-col PSUM tile)
  ACT: zw = Sqrt(scale*c + bias) evacuating PSUM -> fp16 zw   [the ONLY ACT
       pass; sqrt table stays loaded -- no table switches at all]
  DVE: exp via fp16-Schraudolph bitcast:  i16 = cvt(K*zw + B) is the fp16
       bit pattern of 2^v*(1+eps(frac)), v = log2e*(zw+p) + S  (S=16 output
       prescale keeps everything fp16-normal; host multiplies by 2^-16).
       tensor_scalar f16->i16 runs in 4x DVE mode (~2.2us per m-tile).
       To kill the +-3% PL mantissa error, "avg" m-tiles compute two
       half-octave-shifted variants and blend:
           out = f16bits(i(v-1)) + 0.70710678 * f16bits(i(v-0.5))
       (the -1024/-512 bias shifts are exact exponent shifts; the blend is
       one scalar_tensor_tensor, 2x mode) -> ~4x lower error. 2 of 8
       m-tiles stay "plain" (single i16 pass) to keep DVE (<57us) under
       ACT (~59us).
  DMA: fp16 out -> HBM (16MB/core; host upcasts and scales).

Engine budget per core/iter: ACT ~59us (bottleneck), DVE ~57us, PE 28-55us,
DMA out ~47us. Predicted L2 rel err ~1.0e-2 (gate 2e-2): fit 2.3e-3 +
fp16 zw 3.3e-3 + avg-Schraudolph 4.1e-3 on 6/8 rows + plain 1.8e-2 on 2/8.

Mode "gamma" kept for A/B: 2 ACT passes (Sqrt + Exp), bf16 out, 132us.
"""

import contextlib

import numpy as np

import concourse.bacc as bacc
import concourse.mybir as mybir
from concourse.tile import TileContext
from concourse.tile_autobufs import add_dep_helper
from concourse.bass_utils import run_bass_kernel_spmd

AF = mybir.ActivationFunctionType
OP = mybir.AluOpType
F32 = mybir.dt.float32
F16 = mybir.dt.float16
I16 = mybir.dt.int16
BF16 = mybir.dt.bfloat16
BFNP = mybir.dt.np(BF16)

N_CORES = 8
N_ROWS, M_COLS, DIM = 8192, 8192, 64
ROWS = N_ROWS // N_CORES          # 1024 rows of x per core
MTILES = ROWS // 128              # 8 partition tiles per core

# minimax fit of the whole exponent: ln out = GAM_P + GAM_R*sqrt(a*u + b),
# u = 200c+200 in [50, 368]; max abs err on the exponent 2.12e-3.
GAM_P = -22.179313758272478
GAM_R = 0.7814668006400919
GAM_SQ_SCALE = 314.6476142409728          # 200*a
GAM_SQ_BIAS = 325.04327804569425          # 200*a + b

LOG2E = 1.4426950408889634
OUT_EXP = 16                              # device out = true out * 2^OUT_EXP
SIG_AVG = 61.75                           # Schraudolph constants (calibrated
SIG_PLAIN = 62.25                         # numerically on the real data)
# unequal-weight blend: out = f16bits(i(v-1+DLT)) + f16bits(i(v-1.5+DLT));
# DLT makes the nominal weights sum to 1, so the blend is a plain
# tensor_tensor ADD (2x DVE mode; scalar_tensor_tensor has NO fast uop).
BLEND_DLT = 0.22844669683638807           # -log2(2^-1 + 2^-1.5)

# delta: fold GAM_R into the sqrt
D_SQ_SCALE = GAM_R * GAM_R * GAM_SQ_SCALE
D_SQ_BIAS = GAM_R * GAM_R * GAM_SQ_BIAS
D_K = 1024.0 * LOG2E
# i(w) = cvt(1024*w + 1024*15 - sigma) = fp16 bits of ~2^w;  v = LOG2E*zw + PB
D_PB = GAM_P * LOG2E + OUT_EXP

MODE = "delta"
PLAIN_TILES = (3, 7)                      # m-tiles using single-pass exp

_cache = {}


def _build_delta(psum_fd=2048, mm_fd=512, zw_bufs=4, i_bufs=4, o_bufs=2,
                 plain=PLAIN_TILES, iters=1, unroll=1, no_out_dma=False,
                 dbg_act_half=False, dbg_pe_half=False, dma_alt=False):
    nc = bacc.Bacc(None, target_bir_lowering=False)
    xs_d = nc.dram_tensor("xs", [2 * DIM, ROWS], BF16, kind="ExternalInput")
    ys_d = nc.dram_tensor("ys", [2 * DIM, M_COLS], BF16, kind="ExternalInput")
    out_d = nc.dram_tensor("out", [ROWS, M_COLS], F16, kind="ExternalOutput")

    b_base = (D_PB + 15.0) * 1024.0
    b_avg1 = b_base - SIG_AVG + 1024.0 * (BLEND_DLT - 1.0)   # i(v-1+DLT)
    b_avg2 = b_base - SIG_AVG + 1024.0 * (BLEND_DLT - 1.5)   # i(v-1.5+DLT)
    b_plain = b_base - SIG_PLAIN                             # i(v)

    with TileContext(nc) as tc:
        with (
            tc.tile_pool(name="inp", bufs=1) as inp,
            tc.tile_pool(name="consts", bufs=1) as consts,
            tc.tile_pool(name="zw", bufs=zw_bufs) as zwpool,
            tc.tile_pool(name="i16", bufs=i_bufs) as ipool,
            tc.tile_pool(name="obf", bufs=o_bufs) as obfpool,
            tc.tile_pool(name="psum", bufs=4096 // psum_fd, space="PSUM") as psum,
        ):
            xs = inp.tile([2 * DIM, ROWS], BF16)
            ys = inp.tile([2 * DIM, M_COLS], BF16)
            nc.sync.dma_start(out=xs[:], in_=xs_d[:])
            for q in range(0, M_COLS, 2048):
                nc.sync.dma_start(out=ys[:, q:q + 2048], in_=ys_d[:, q:q + 2048])

            bsq = consts.tile([128, 1], F32)
            nc.gpsimd.memset(bsq[:], float(D_SQ_BIAS))

            nchunk = M_COLS // psum_fd
            loop_cm = tc.For_i(0, iters) if iters > 1 else contextlib.nullcontext(0)
            with loop_cm as _i:
              for _u in range(unroll):
                for m in range(MTILES):
                    msl = slice(m * 128, (m + 1) * 128)
                    zw = zwpool.tile([128, M_COLS], F16, tag="zw")
                    for nb in range(nchunk):
                        pt = psum.tile([128, psum_fd], F32, tag="ps")
                        nmm = psum_fd // mm_fd
                        if dbg_pe_half:
                            nmm //= 2
                        for j in range(nmm):
                            col = nb * psum_fd + j * mm_fd
                            nc.tensor.matmul(
                                pt[:, j * mm_fd:(j + 1) * mm_fd],
                                xs[:, msl], ys[:, col:col + mm_fd],
                                start=True, stop=True,
                            )
                        sl = slice(nb * psum_fd, (nb + 1) * psum_fd)
                        if dbg_act_half:
                            nc.scalar.activation(
                                zw[:, nb * psum_fd:nb * psum_fd + psum_fd // 2],
                                pt[:, :psum_fd // 2], AF.Sqrt,
                                bias=bsq[:], scale=float(D_SQ_SCALE),
                            )
                        else:
                            nc.scalar.activation(
                                zw[:, sl], pt[:], AF.Sqrt,
                                bias=bsq[:], scale=float(D_SQ_SCALE),
                            )
                    osl = slice(m * 128, (m + 1) * 128)
                    if m in plain:
                        i1 = ipool.tile([128, M_COLS], I16, tag="i16")
                        nc.vector.tensor_scalar(
                            out=i1[:], in0=zw[:],
                            scalar1=float(D_K), scalar2=float(b_plain),
                            op0=OP.mult, op1=OP.add,
                        )
                        if not no_out_dma:
                            deng = nc.scalar if (dma_alt and m % 2) else nc.sync
                            deng.dma_start(out=out_d[osl, :], in_=i1[:].bitcast(F16))
                    else:
                        i1 = ipool.tile([128, M_COLS], I16, tag="i16")
                        i2 = ipool.tile([128, M_COLS], I16, tag="i16")
                        nc.vector.tensor_scalar(
                            out=i1[:], in0=zw[:],
                            scalar1=float(D_K), scalar2=float(b_avg1),
                            op0=OP.mult, op1=OP.add,
                        )
                        nc.vector.tensor_scalar(
                            out=i2[:], in0=zw[:],
                            scalar1=float(D_K), scalar2=float(b_avg2),
                            op0=OP.mult, op1=OP.add,
                        )
                        obf = obfpool.tile([128, M_COLS], F16, tag="obf")
                        nc.vector.tensor_tensor(
                            obf[:], i1[:].bitcast(F16), i2[:].bitcast(F16),
                            OP.add,
                        )
                        if not no_out_dma:
                            deng = nc.scalar if (dma_alt and m % 2) else nc.sync
                            deng.dma_start(out=out_d[osl, :], in_=obf[:])

    nc.finalize()
    return nc


def _build_gamma(group=8, psum_fd=1024, obf_bufs=3, zw_extra=0, exp_split=1,
                 iters=1, zw_fp16=True, no_yl=True, mm_fd=512):
    """mode gamma (baseline): 2 ACT passes, table-set batched."""
    nc = bacc.Bacc(None, target_bir_lowering=False)
    xs_d = nc.dram_tensor("xs", [2 * DIM, ROWS], BF16, kind="ExternalInput")
    ys_d = nc.dram_tensor("ys", [2 * DIM, M_COLS], BF16, kind="ExternalInput")
    out_d = nc.dram_tensor("out", [ROWS, M_COLS], BF16, kind="ExternalOutput")

    zw_dt = mybir.dt.float16 if zw_fp16 else F32

    with TileContext(nc) as tc:
        with (
            tc.tile_pool(name="inp", bufs=1) as inp,
            tc.tile_pool(name="consts", bufs=1) as consts,
            tc.tile_pool(name="zw", bufs=group + zw_extra) as zwpool,
            tc.tile_pool(name="obf", bufs=obf_bufs) as obfpool,
            tc.tile_pool(name="psum", bufs=4096 // psum_fd, space="PSUM") as psum,
        ):
            xs = inp.tile([2 * DIM, ROWS], BF16)
            ys = inp.tile([2 * DIM, M_COLS], BF16)
            nc.sync.dma_start(out=xs[:], in_=xs_d[:])
            for q in range(0, M_COLS, 2048):
                nc.sync.dma_start(out=ys[:, q:q + 2048], in_=ys_d[:, q:q + 2048])

            bsq = consts.tile([128, 1], F32)
            nc.gpsimd.memset(bsq[:], float(GAM_SQ_BIAS))
            bexp = consts.tile([128, 1], F32)
            nc.gpsimd.memset(bexp[:], float(GAM_P))

            nchunk = M_COLS // psum_fd
            mtile_groups = [
                list(range(g, min(g + group, MTILES)))
                for g in range(0, MTILES, group)
            ]
            loop_cm = tc.For_i(0, iters) if iters > 1 else contextlib.nullcontext(0)
            with loop_cm as _i:
                last_exp = None
                for grp in mtile_groups:
                    zw_tiles = {}
                    last_evac = None
                    for m in grp:
                        zw = zwpool.tile([128, M_COLS], zw_dt, tag="zw")
                        zw_tiles[m] = zw
                        msl = slice(m * 128, (m + 1) * 128)
                        for nb in range(nchunk):
                            pt = psum.tile([128, psum_fd], F32, tag="ps")
                            for j in range(psum_fd // mm_fd):
                                col = nb * psum_fd + j * mm_fd
                                jsl = slice(j * mm_fd, (j + 1) * mm_fd)
                                nc.tensor.matmul(
                                    pt[:, jsl],
                                    xs[:, msl], ys[:, col:col + mm_fd],
                                    start=True, stop=True,
                                )
                            sl = slice(nb * psum_fd, (nb + 1) * psum_fd)
                            ev = nc.scalar.activation(
                                zw[:, sl], pt[:], AF.Sqrt,
                                bias=bsq[:], scale=float(GAM_SQ_SCALE)
                            )
                            if last_exp is not None:
                                add_dep_helper(
                                    ev.ins, last_exp.ins, sync=False,
                                    reason="batch sqrt after prev group exp",
                                )
                            last_evac = ev
                    for m in grp:
                        zw = zw_tiles[m]
                        efd = M_COLS // exp_split
                        obf = obfpool.tile([128, M_COLS], BF16, tag="obf")
                        for e in range(exp_split):
                            esl = slice(e * efd, (e + 1) * efd)
                            exp_inst = nc.scalar.activation(
                                obf[:, esl], zw[:, esl], AF.Exp,
                                bias=bexp[:], scale=float(GAM_R)
                            )
                            add_dep_helper(
                                exp_inst.ins, last_evac.ins, sync=False,
                                reason="batch exp after group sqrt (table switch)",
                            )
                            last_exp = exp_inst
                            nc.sync.dma_start(
                                out=out_d[m * 128:(m + 1) * 128, esl],
                                in_=obf[:, esl],
                            )

    nc.finalize()
    return nc


def _build(mode, iters=1, **kw):
    if mode == "delta":
        return _build_delta(iters=iters, **kw)
    return _build_gamma(iters=iters, **kw)


LAST_RESULTS = None


def _split_bf16(a):
    hi = a.astype(BFNP)
    lo = (a - hi.astype(np.float32)).astype(BFNP)
    return hi, lo


def make_in_maps(x, y):
    yT = y.T
    yh, _yl = _split_bf16(yT)
    ys = np.ascontiguousarray(np.concatenate([yh, yh], axis=0))
    in_maps = []
    for i in range(N_CORES):
        xT = x[i * ROWS:(i + 1) * ROWS].T
        xh, xl = _split_bf16(xT)
        xstack = np.ascontiguousarray(np.concatenate([xh, xl], axis=0))
        in_maps.append({"xs": xstack, "ys": ys})
    return in_maps


def kernel(x: np.ndarray, y: np.ndarray) -> np.ndarray:
    global LAST_RESULTS
    x = np.ascontiguousarray(x, dtype=np.float32)
    y = np.ascontiguousarray(y, dtype=np.float32)
    assert x.shape == (N_ROWS, DIM) and y.shape == (M_COLS, DIM)

    if MODE not in _cache:
        _cache[MODE] = _build(MODE)
    nc = _cache[MODE]

    in_maps = make_in_maps(x, y)

    LAST_RESULTS = run_bass_kernel_spmd(nc, in_maps, list(range(N_CORES)))
    out = np.concatenate([r["out"] for r in LAST_RESULTS.results], axis=0)
    if out.dtype == np.float16:
        out = out.astype(np.float32) * np.float32(2.0 ** -OUT_EXP)
    elif out.dtype != np.float32:
        out = out.astype(np.float32)
    return out


# revision 17
# speedup vs baseline: 1.5788x; 1.0890x over previous
"""Trainium2 Bass kernel for nn_Bessel: out = i0e(z) * exp(z - 2a), z = 2a*sqrt((1+x@yT)/2), a=10.

Mode "delta" (current): single ACT pass + DVE bitcast-exp.

Math: ln out ~= p + r*sqrt(SQ*c + SB)  (4-param minimax fit, c = x@yT).
Fold r into the sqrt:  zw = sqrt(r^2*SQ*c + r^2*SB),  out = exp(zw + p).

Per core (row-shard of x, y replicated; out tile [1024, 8192]):

  PE:  c into PSUM as [xh;xl] @ [yh;yh]  (bf16 split of x only; K=128,
       4 matmuls of 512 cols per 2048-col PSUM tile)
  ACT: zw = Sqrt(scale*c + bias) evacuating PSUM -> fp16 zw   [the ONLY ACT
       pass; sqrt table stays loaded -- no table switches at all]
  DVE: exp via fp16-Schraudolph bitcast:  i16 = cvt(K*zw + B) is the fp16
       bit pattern of 2^v*(1+eps(frac)), v = log2e*(zw+p) + S  (S=16 output
       prescale keeps everything fp16-normal; host multiplies by 2^-16).
       tensor_scalar f16->i16 runs in 4x DVE mode (~2.2us per m-tile).
       To kill the +-3% PL mantissa error, "avg" m-tiles compute two
       half-octave-shifted variants and blend:
           out = f16bits(i(v-1+DLT)) + f16bits(i(v-1.5+DLT))
       where DLT = -log2(2^-1 + 2^-1.5) makes the nominal weights sum to 1,
       so the blend is a plain tensor_tensor ADD (2x mode; NB
       scalar_tensor_tensor has NO fast uop -- modes [] -- and runs 1x).
       The half-octave phase shift between the two terms cancels ~4x of the
       PL error (residual ~0.6%). 2 of 8 m-tiles stay "plain" (single i16
       pass) to keep DVE busy (~57us) under ACT (~59us).
  DMA: fp16 out -> HBM (16MB/core; host upcasts and scales by 2^-16).

Engine budget per core/iter: ACT ~59us (bottleneck), DVE ~57us, PE 28-55us,
DMA out ~47us.  Measured 66-68us (vs 132.3us baseline, ~1.95x): the extra
~7us over the ACT floor is distributed sem/pipeline overhead (~4us) + out-DMA
coupling (~3us; no_out_dma measures 61-63us = TimelineSim's 60.5).  Tuning
notes (HW-benched): psum 2048x2 > 1024x4; zw_bufs=4 (cross-For_i-iteration
wrap slack; zw3 and zw5 no better); i_bufs=4 and o_bufs=2 are local optima
(i5/o3 regress ~10us -- SBUF bank placement sensitivity); issuing out-DMAs on
the scalar queue (dma_alt) stalls ACT, +18us; splitting DVE/DMA per half-tile
is neutral.  Timing uses unroll=4 bodies per For_i iteration: the loop
back-edge drains the pipeline (~16us/iter at unroll=1, amortized away by 4).

L2 rel err 1.0607e-2 (gate 2e-2): fit 2.3e-3 + fp16 zw -> 4.0e-3, +
blend-Schraudolph 6.4e-3 on 6/8 rows, plain 1.8e-2 on 2/8 rows.

Mode "gamma" kept for A/B: 2 ACT passes (Sqrt + Exp), bf16 out, 132us.
"""

import contextlib

import numpy as np

import concourse.bacc as bacc
import concourse.mybir as mybir
from concourse.tile import TileContext
from concourse.tile_autobufs import add_dep_helper
from concourse.bass_utils import run_bass_kernel_spmd

AF = mybir.ActivationFunctionType
OP = mybir.AluOpType
F32 = mybir.dt.float32
F16 = mybir.dt.float16
I16 = mybir.dt.int16
BF16 = mybir.dt.bfloat16
BFNP = mybir.dt.np(BF16)

N_CORES = 8
N_ROWS, M_COLS, DIM = 8192, 8192, 64
ROWS = N_ROWS // N_CORES          # 1024 rows of x per core
MTILES = ROWS // 128              # 8 partition tiles per core

# minimax fit of the whole exponent: ln out = GAM_P + GAM_R*sqrt(a*u + b),
# u = 200c+200 in [50, 368]; max abs err on the exponent 2.12e-3.
GAM_P = -22.179313758272478
GAM_R = 0.7814668006400919
GAM_SQ_SCALE = 314.6476142409728          # 200*a
GAM_SQ_BIAS = 325.04327804569425          # 200*a + b

LOG2E = 1.4426950408889634
OUT_EXP = 16                              # device out = true out * 2^OUT_EXP
SIG_AVG = 61.75                           # Schraudolph constants (calibrated
SIG_PLAIN = 62.25                         # numerically on the real data)
# unequal-weight blend: out = f16bits(i(v-1+DLT)) + f16bits(i(v-1.5+DLT));
# DLT makes the nominal weights sum to 1, so the blend is a plain
# tensor_tensor ADD (2x DVE mode; scalar_tensor_tensor has NO fast uop).
BLEND_DLT = 0.22844669683638807           # -log2(2^-1 + 2^-1.5)

# delta: fold GAM_R into the sqrt
D_SQ_SCALE = GAM_R * GAM_R * GAM_SQ_SCALE
D_SQ_BIAS = GAM_R * GAM_R * GAM_SQ_BIAS
D_K = 1024.0 * LOG2E
# i(w) = cvt(1024*w + 1024*15 - sigma) = fp16 bits of ~2^w;  v = LOG2E*zw + PB
D_PB = GAM_P * LOG2E + OUT_EXP

MODE = "delta"
PLAIN_TILES = (3, 7)                      # m-tiles using single-pass exp

_cache = {}


def _build_delta(psum_fd=2048, mm_fd=512, zw_bufs=4, i_bufs=4, o_bufs=2,
                 plain=PLAIN_TILES, iters=1, unroll=1, no_out_dma=False,
                 dbg_act_half=False, dbg_pe_half=False, dma_alt=False,
                 dve_split=1):
    nc = bacc.Bacc(None, target_bir_lowering=False)
    xs_d = nc.dram_tensor("xs", [2 * DIM, ROWS], BF16, kind="ExternalInput")
    ys_d = nc.dram_tensor("ys", [2 * DIM, M_COLS], BF16, kind="ExternalInput")
    out_d = nc.dram_tensor("out", [ROWS, M_COLS], F16, kind="ExternalOutput")

    b_base = (D_PB + 15.0) * 1024.0
    b_avg1 = b_base - SIG_AVG + 1024.0 * (BLEND_DLT - 1.0)   # i(v-1+DLT)
    b_avg2 = b_base - SIG_AVG + 1024.0 * (BLEND_DLT - 1.5)   # i(v-1.5+DLT)
    b_plain = b_base - SIG_PLAIN                             # i(v)

    with TileContext(nc) as tc:
        with (
            tc.tile_pool(name="inp", bufs=1) as inp,
            tc.tile_pool(name="consts", bufs=1) as consts,
            tc.tile_pool(name="zw", bufs=zw_bufs) as zwpool,
            tc.tile_pool(name="i16", bufs=i_bufs) as ipool,
            tc.tile_pool(name="obf", bufs=o_bufs) as obfpool,
            tc.tile_pool(name="psum", bufs=4096 // psum_fd, space="PSUM") as psum,
        ):
            xs = inp.tile([2 * DIM, ROWS], BF16)
            ys = inp.tile([2 * DIM, M_COLS], BF16)
            nc.sync.dma_start(out=xs[:], in_=xs_d[:])
            for q in range(0, M_COLS, 2048):
                nc.sync.dma_start(out=ys[:, q:q + 2048], in_=ys_d[:, q:q + 2048])

            bsq = consts.tile([128, 1], F32)
            nc.gpsimd.memset(bsq[:], float(D_SQ_BIAS))

            nchunk = M_COLS // psum_fd
            loop_cm = tc.For_i(0, iters) if iters > 1 else contextlib.nullcontext(0)
            with loop_cm as _i:
              for _u in range(unroll):
                for m in range(MTILES):
                    msl = slice(m * 128, (m + 1) * 128)
                    zw = zwpool.tile([128, M_COLS], F16, tag="zw")
                    for nb in range(nchunk):
                        pt = psum.tile([128, psum_fd], F32, tag="ps")
                        nmm = psum_fd // mm_fd
                        if dbg_pe_half:
                            nmm //= 2
                        for j in range(nmm):
                            col = nb * psum_fd + j * mm_fd
                            nc.tensor.matmul(
                                pt[:, j * mm_fd:(j + 1) * mm_fd],
                                xs[:, msl], ys[:, col:col + mm_fd],
                                start=True, stop=True,
                            )
                        sl = slice(nb * psum_fd, (nb + 1) * psum_fd)
                        if dbg_act_half:
                            nc.scalar.activation(
                                zw[:, nb * psum_fd:nb * psum_fd + psum_fd // 2],
                                pt[:, :psum_fd // 2], AF.Sqrt,
                                bias=bsq[:], scale=float(D_SQ_SCALE),
                            )
                        else:
                            nc.scalar.activation(
                                zw[:, sl], pt[:], AF.Sqrt,
                                bias=bsq[:], scale=float(D_SQ_SCALE),
                            )
                    osl = slice(m * 128, (m + 1) * 128)
                    hfd = M_COLS // dve_split
                    if m in plain:
                        i1 = ipool.tile([128, M_COLS], I16, tag="i16")
                        for h in range(dve_split):
                            hsl = slice(h * hfd, (h + 1) * hfd)
                            nc.vector.tensor_scalar(
                                out=i1[:, hsl], in0=zw[:, hsl],
                                scalar1=float(D_K), scalar2=float(b_plain),
                                op0=OP.mult, op1=OP.add,
                            )
                            if not no_out_dma:
                                deng = nc.scalar if (dma_alt and m % 2) else nc.sync
                                deng.dma_start(out=out_d[osl, hsl],
                                               in_=i1[:, hsl].bitcast(F16))
                    else:
                        i1 = ipool.tile([128, M_COLS], I16, tag="i16")
                        i2 = ipool.tile([128, M_COLS], I16, tag="i16")
                        obf = obfpool.tile([128, M_COLS], F16, tag="obf")
                        for h in range(dve_split):
                            hsl = slice(h * hfd, (h + 1) * hfd)
                            nc.vector.tensor_scalar(
                                out=i1[:, hsl], in0=zw[:, hsl],
                                scalar1=float(D_K), scalar2=float(b_avg1),
                                op0=OP.mult, op1=OP.add,
                            )
                            nc.vector.tensor_scalar(
                                out=i2[:, hsl], in0=zw[:, hsl],
                                scalar1=float(D_K), scalar2=float(b_avg2),
                                op0=OP.mult, op1=OP.add,
                            )
                            nc.vector.tensor_tensor(
                                obf[:, hsl], i1[:, hsl].bitcast(F16),
                                i2[:, hsl].bitcast(F16), OP.add,
                            )
                            if not no_out_dma:
                                deng = nc.scalar if (dma_alt and m % 2) else nc.sync
                                deng.dma_start(out=out_d[osl, hsl], in_=obf[:, hsl])

    nc.finalize()
    return nc


def _build_gamma(group=8, psum_fd=1024, obf_bufs=3, zw_extra=0, exp_split=1,
                 iters=1, zw_fp16=True, no_yl=True, mm_fd=512):
    """mode gamma (baseline): 2 ACT passes, table-set batched."""
    nc = bacc.Bacc(None, target_bir_lowering=False)
    xs_d = nc.dram_tensor("xs", [2 * DIM, ROWS], BF16, kind="ExternalInput")
    ys_d = nc.dram_tensor("ys", [2 * DIM, M_COLS], BF16, kind="ExternalInput")
    out_d = nc.dram_tensor("out", [ROWS, M_COLS], BF16, kind="ExternalOutput")

    zw_dt = mybir.dt.float16 if zw_fp16 else F32

    with TileContext(nc) as tc:
        with (
            tc.tile_pool(name="inp", bufs=1) as inp,
            tc.tile_pool(name="consts", bufs=1) as consts,
            tc.tile_pool(name="zw", bufs=group + zw_extra) as zwpool,
            tc.tile_pool(name="obf", bufs=obf_bufs) as obfpool,
            tc.tile_pool(name="psum", bufs=4096 // psum_fd, space="PSUM") as psum,
        ):
            xs = inp.tile([2 * DIM, ROWS], BF16)
            ys = inp.tile([2 * DIM, M_COLS], BF16)
            nc.sync.dma_start(out=xs[:], in_=xs_d[:])
            for q in range(0, M_COLS, 2048):
                nc.sync.dma_start(out=ys[:, q:q + 2048], in_=ys_d[:, q:q + 2048])

            bsq = consts.tile([128, 1], F32)
            nc.gpsimd.memset(bsq[:], float(GAM_SQ_BIAS))
            bexp = consts.tile([128, 1], F32)
            nc.gpsimd.memset(bexp[:], float(GAM_P))

            nchunk = M_COLS // psum_fd
            mtile_groups = [
                list(range(g, min(g + group, MTILES)))
                for g in range(0, MTILES, group)
            ]
            loop_cm = tc.For_i(0, iters) if iters > 1 else contextlib.nullcontext(0)
            with loop_cm as _i:
                last_exp = None
                for grp in mtile_groups:
                    zw_tiles = {}
                    last_evac = None
                    for m in grp:
                        zw = zwpool.tile([128, M_COLS], zw_dt, tag="zw")
                        zw_tiles[m] = zw
                        msl = slice(m * 128, (m + 1) * 128)
                        for nb in range(nchunk):
                            pt = psum.tile([128, psum_fd], F32, tag="ps")
                            for j in range(psum_fd // mm_fd):
                                col = nb * psum_fd + j * mm_fd
                                jsl = slice(j * mm_fd, (j + 1) * mm_fd)
                                nc.tensor.matmul(
                                    pt[:, jsl],
                                    xs[:, msl], ys[:, col:col + mm_fd],
                                    start=True, stop=True,
                                )
                            sl = slice(nb * psum_fd, (nb + 1) * psum_fd)
                            ev = nc.scalar.activation(
                                zw[:, sl], pt[:], AF.Sqrt,
                                bias=bsq[:], scale=float(GAM_SQ_SCALE)
                            )
                            if last_exp is not None:
                                add_dep_helper(
                                    ev.ins, last_exp.ins, sync=False,
                                    reason="batch sqrt after prev group exp",
                                )
                            last_evac = ev
                    for m in grp:
                        zw = zw_tiles[m]
                        efd = M_COLS // exp_split
                        obf = obfpool.tile([128, M_COLS], BF16, tag="obf")
                        for e in range(exp_split):
                            esl = slice(e * efd, (e + 1) * efd)
                            exp_inst = nc.scalar.activation(
                                obf[:, esl], zw[:, esl], AF.Exp,
                                bias=bexp[:], scale=float(GAM_R)
                            )
                            add_dep_helper(
                                exp_inst.ins, last_evac.ins, sync=False,
                                reason="batch exp after group sqrt (table switch)",
                            )
                            last_exp = exp_inst
                            nc.sync.dma_start(
                                out=out_d[m * 128:(m + 1) * 128, esl],
                                in_=obf[:, esl],
                            )

    nc.finalize()
    return nc


def _build(mode, iters=1, **kw):
    if mode == "delta":
        return _build_delta(iters=iters, **kw)
    return _build_gamma(iters=iters, **kw)


LAST_RESULTS = None


def _split_bf16(a):
    hi = a.astype(BFNP)
    lo = (a - hi.astype(np.float32)).astype(BFNP)
    return hi, lo


def make_in_maps(x, y):
    yT = y.T
    yh, _yl = _split_bf16(yT)
    ys = np.ascontiguousarray(np.concatenate([yh, yh], axis=0))
    in_maps = []
    for i in range(N_CORES):
        xT = x[i * ROWS:(i + 1) * ROWS].T
        xh, xl = _split_bf16(xT)
        xstack = np.ascontiguousarray(np.concatenate([xh, xl], axis=0))
        in_maps.append({"xs": xstack, "ys": ys})
    return in_maps


def kernel(x: np.ndarray, y: np.ndarray) -> np.ndarray:
    global LAST_RESULTS
    x = np.ascontiguousarray(x, dtype=np.float32)
    y = np.ascontiguousarray(y, dtype=np.float32)
    assert x.shape == (N_ROWS, DIM) and y.shape == (M_COLS, DIM)

    if MODE not in _cache:
        _cache[MODE] = _build(MODE)
    nc = _cache[MODE]

    in_maps = make_in_maps(x, y)

    LAST_RESULTS = run_bass_kernel_spmd(nc, in_maps, list(range(N_CORES)))
    out = np.concatenate([r["out"] for r in LAST_RESULTS.results], axis=0)
    if out.dtype == np.float16:
        out = out.astype(np.float32) * np.float32(2.0 ** -OUT_EXP)
    elif out.dtype != np.float32:
        out = out.astype(np.float32)
    return out
